# revision 1
# baseline (speedup 1.0000x reference)
"""AdaptiveLSTMCellWithRes on 8 TRN2 NeuronCores.

Data-parallel over batch (1024 rows/core), weights replicated.
All on-chip compute happens in transposed-activation space [feat, batch]:
  - host pre-packs each weight matrix into stationary-tile slabs
    pack[j, p, k*128+m] = W[j*128+m, k*128+p]  (so W^T tiles load contiguously)
  - host pre-transposes x/h_prev/c_prev, un-transposes outputs
  - gate matmuls fuse W@x + U@h into one K=2048 accumulation over concat(x,h)
  - biases fold into the ScalarE activation that evicts PSUM
Matmuls run as float32r (full-rate fp32 on the PE array).
"""

import sys

if "/opt/trn_rl_repo" not in sys.path:
    sys.path.insert(0, "/opt/trn_rl_repo")

import numpy as np

P = 128
B = 8192          # global batch
NCORES = 8
BL = B // NCORES  # batch per core (1024)
D = 1024          # feature dim
K2 = 2048         # concat(x, h) contraction
JC = D // P       # 8 output-feature tiles
KC2 = K2 // P     # 16 k-chunks for gates/a1
KC1 = D // P      # 8 k-chunks for residual/a2
NH = BL // 2      # moving free dim per matmul (512)

_CACHE = {}


def _build():
    import concourse.bass as bass  # noqa: F401
    from concourse import bacc, mybir
    import concourse.tile as tile

    F32 = mybir.dt.float32
    MMDT = mybir.dt.float32r
    AF = mybir.ActivationFunctionType

    nc = bacc.Bacc()

    # gate weights (i, f, o, c, s, a1): packed [6, JC, P, K2]
    wg6 = nc.declare_dram_parameter("wg6", [6, JC, P, K2], MMDT, isOutput=False)
    # residual weights (r1, r2, r3): packed [3, JC, P, D]
    wr = nc.declare_dram_parameter("wr", [3, JC, P, D], MMDT, isOutput=False)
    # a2 weight: [P, KC1] with a2p[p, k] = a2_w[0, k*128+p]
    a2p = nc.declare_dram_parameter("a2p", [P, KC1], MMDT, isOutput=False)
    # biases: [P, 10*JC]; col v*JC+j holds vec_v[j*128:(j+1)*128]
    # v: 0..4 = combined gate biases (i,f,o,c,s), 5=a1_b, 6=r1_b, 7=r2_b,
    # 8=r3_b, 9=a2_b (replicated)
    biasp = nc.declare_dram_parameter("biasp", [P, 10 * JC], F32, isOutput=False)
    # transposed activations: rows 0..D-1 = x^T, D..2D-1 = h^T
    xhT = nc.declare_dram_parameter("xhT", [K2, BL], MMDT, isOutput=False)
    cT = nc.declare_dram_parameter("cT", [D, BL], F32, isOutput=False)
    # out[0] = h_t^T, out[1] = c_t^T
    out = nc.declare_dram_parameter("out", [2, D, BL], F32, isOutput=True)

    alpha_dram = nc.dram_tensor("alpha_dram", [1, BL], F32)

    GATE_FN = [AF.Sigmoid, AF.Sigmoid, AF.Sigmoid, AF.Tanh, AF.Sigmoid]

    with tile.TileContext(nc) as tc:
        with (
            tc.tile_pool(name="consts", bufs=1) as consts,
            tc.tile_pool(name="xh", bufs=1) as xh_pool,
            tc.tile_pool(name="w", bufs=5) as w_pool,
            tc.tile_pool(name="a1s", bufs=4) as a1_pool,
            tc.tile_pool(name="r1", bufs=1) as r1_pool,
            tc.tile_pool(name="r2", bufs=1) as r2_pool,
            tc.tile_pool(name="gates", bufs=1) as g_pool,
            tc.tile_pool(name="ew", bufs=2) as ew_pool,
            tc.tile_pool(name="psum", bufs=3, space="PSUM") as psum_pool,
            tc.tile_pool(name="psum_a2", bufs=1, space="PSUM") as psum_a2_pool,
        ):
            bias_sb = consts.tile([P, 10 * JC], F32, name="bias_sb")
            a2_sb = consts.tile([P, KC1], MMDT, name="a2_sb")

            def bias_ap(v, j):
                return bias_sb[:, v * JC + j: v * JC + j + 1]

            # h-half first: phase A's r1 only needs xh[KC1:]
            xh = [None] * KC2

            def load_xh(k):
                t = xh_pool.tile([P, BL], MMDT, tag=f"xh{k}", name=f"xh{k}")
                nc.sync.dma_start(out=t[:], in_=xhT[k * P:(k + 1) * P, :])
                xh[k] = t


            def mm_pair(ps2, wslabs, rhs_tiles, kc):
                # k outer / bh inner: each stationary tile feeds 2 matmuls
                for k in range(kc):
                    wt = wslabs[k // KC1]
                    kk = k % KC1
                    for bh in range(2):
                        mv = slice(bh * NH, (bh + 1) * NH)
                        nc.tensor.matmul(
                            ps2[bh][:], wt[:, kk * P:(kk + 1) * P],
                            rhs_tiles[k][:, mv],
                            start=(k == 0), stop=(k == kc - 1))

            def load_w(src_ap2, nslabs, name):
                slabs = []
                for i in range(nslabs):
                    wt = w_pool.tile([P, D], MMDT, tag="w", name=f"{name}{i}")
                    nc.sync.dma_start(out=wt[:], in_=src_ap2[:, i * D:(i + 1) * D])
                    slabs.append(wt)
                return slabs

            # first two r1 weight slabs must beat the xh stream so the PE
            # can start as soon as the first h tiles land
            r1w_pre = [load_w(wr[0, 0], 1, "wt_r1p0"),
                       load_w(wr[0, 1], 1, "wt_r1p1")]
            for k in range(KC1, KC2):
                load_xh(k)
            # consts aren't needed until the first PSUM eviction (~17us);
            # keep them out of the DMA queues' critical prefix
            nc.sync.dma_start(out=bias_sb[:], in_=biasp[:, :])
            nc.sync.dma_start(out=a2_sb[:], in_=a2p[:, :])


            # ---- phase A: r1 (only needs h-half of xh); a1 -> a2; r2 ----
            r1 = []
            for j in range(JC):
                ws = r1w_pre[j] if j < 2 else load_w(wr[0, j], 1, "wt_r1")
                t = r1_pool.tile([P, BL], MMDT, tag=f"r1_{j}", name=f"r1_{j}")
                ps2 = [psum_pool.tile([P, NH], F32, tag="ps0", name="ps_r1_0"),
                       psum_pool.tile([P, NH], F32, tag="ps1", name="ps_r1_1")]
                mm_pair(ps2, ws, xh[KC1:], KC1)
                for bh in range(2):
                    nc.scalar.activation(t[:, bh * NH:(bh + 1) * NH], ps2[bh][:],
                                         AF.Relu, bias=bias_ap(6, j))
                r1.append(t)

            # x-half loads overlap r1 compute
            for k in range(KC1):
                load_xh(k)

            ps_a2 = [psum_a2_pool.tile([1, NH], F32, tag="a20", name="psa20"),
                     psum_a2_pool.tile([1, NH], F32, tag="a21", name="psa21")]
            pend = []

            def flush_a2():
                jq, pair = pend.pop(0)
                for bh in range(2):
                    nc.tensor.matmul(ps_a2[bh][:], a2_sb[:, jq:jq + 1],
                                     pair[bh][:], start=(jq == 0),
                                     stop=(jq == JC - 1))

            for j in range(JC):
                ws = load_w(wg6[5, j], 2, "wt_a1")
                ps2 = [psum_pool.tile([P, NH], F32, tag="ps0", name="ps_a1_0"),
                       psum_pool.tile([P, NH], F32, tag="ps1", name="ps_a1_1")]
                mm_pair(ps2, ws, xh, KC2)
                pair = []
                for bh in range(2):
                    a1b = a1_pool.tile([P, NH], MMDT, tag="a1", name="a1b")
                    nc.scalar.activation(a1b[:], ps2[bh][:], AF.Relu,
                                         bias=bias_ap(5, j))
                    pair.append(a1b)
                pend.append((j, pair))
                # defer the tiny a2 matmuls one j so PE never waits on ScalarE
                if len(pend) == 2:
                    flush_a2()
            while pend:
                flush_a2()

            r2 = []
            for j in range(JC):
                ws = load_w(wr[1, j], 1, "wt_r2")
                t = r2_pool.tile([P, BL], MMDT, tag=f"r2_{j}", name=f"r2_{j}")
                ps2 = [psum_pool.tile([P, NH], F32, tag="ps0", name="ps_r2_0"),
                       psum_pool.tile([P, NH], F32, tag="ps1", name="ps_r2_1")]
                mm_pair(ps2, ws, r1, KC1)
                for bh in range(2):
                    nc.scalar.activation(t[:, bh * NH:(bh + 1) * NH], ps2[bh][:],
                                         AF.Relu, bias=bias_ap(7, j))
                r2.append(t)

            # alpha = sigmoid(a2 @ a1relu + a2_b): [1, BL]; broadcast via DRAM
            for bh in range(2):
                asb = a1_pool.tile([1, NH], F32, tag="a1", name="alpha_sb")
                nc.scalar.activation(asb[:], ps_a2[bh][:], AF.Sigmoid,
                                     bias=bias_sb[0:1, 9 * JC: 9 * JC + 1])
                nc.sync.dma_start(out=alpha_dram[0:1, bh * NH:(bh + 1) * NH],
                                  in_=asb[:])
            alpha_rep = consts.tile([P, BL], F32, name="alpha_rep")
            nc.gpsimd.dma_start(
                out=alpha_rep[:], in_=alpha_dram[0:1, :].broadcast_to([P, BL]))

            # ---- phase B: gates + r3 + combine, per feature tile j.
            # Gate order c,s,i,f,r3,o lets the elementwise chain run while
            # later matmuls stream, so only h=o*tanh(c) trails the last MM.
            def gate_mm(g, j):
                ws = load_w(wg6[g, j], 2, f"wt_g{g}")
                t = g_pool.tile([P, BL], F32, tag=f"g{g}", name=f"g{g}")
                ps2 = [psum_pool.tile([P, NH], F32, tag="ps0", name="ps_g0"),
                       psum_pool.tile([P, NH], F32, tag="ps1", name="ps_g1")]
                mm_pair(ps2, ws, xh, KC2)
                for bh in range(2):
                    nc.scalar.activation(t[:, bh * NH:(bh + 1) * NH],
                                         ps2[bh][:], GATE_FN[g],
                                         bias=bias_ap(g, j))
                return t

            for j in range(JC):
                ch = gate_mm(3, j)
                st = gate_mm(4, j)
                it = gate_mm(0, j)

                t1s, t2s, ths = [], [], []
                for bh in range(2):
                    mv = slice(bh * NH, (bh + 1) * NH)
                    t1 = ew_pool.tile([P, NH], F32, tag=f"t1{bh}", name="t1")
                    nc.vector.tensor_mul(t1[:], it[:, mv], ch[:, mv])
                    nc.vector.tensor_mul(t1[:], t1[:], st[:, mv])
                    nc.vector.tensor_mul(t1[:], t1[:], alpha_rep[:, mv])
                    t1s.append(t1)

                ft = gate_mm(1, j)
                for bh in range(2):
                    mv = slice(bh * NH, (bh + 1) * NH)
                    cp = ew_pool.tile([P, NH], F32, tag="cp", name="cp", bufs=1)
                    nc.sync.dma_start(out=cp[:], in_=cT[j * P:(j + 1) * P, mv])
                    t2 = ew_pool.tile([P, NH], F32, tag=f"t2{bh}", name="t2")
                    nc.vector.tensor_mul(t2[:], ft[:, mv], cp[:])
                    nc.vector.tensor_add(t1s[bh][:], t1s[bh][:], t2[:])
                    t2s.append(t2)

                ws = load_w(wr[2, j], 1, "wt_r3")
                r3 = g_pool.tile([P, BL], F32, tag="r3", name="r3")
                ps2 = [psum_pool.tile([P, NH], F32, tag="ps0", name="ps_r3_0"),
                       psum_pool.tile([P, NH], F32, tag="ps1", name="ps_r3_1")]
                mm_pair(ps2, ws, r2, KC1)
                for bh in range(2):
                    nc.scalar.activation(r3[:, bh * NH:(bh + 1) * NH], ps2[bh][:],
                                         AF.Identity, bias=bias_ap(8, j))
                for bh in range(2):
                    mv = slice(bh * NH, (bh + 1) * NH)
                    nc.vector.tensor_add(t1s[bh][:], t1s[bh][:], r3[:, mv])
                    th = ew_pool.tile([P, NH], F32, tag=f"th{bh}", name="th",
                                      bufs=1)
                    nc.scalar.activation(th[:], t1s[bh][:], AF.Tanh)
                    ths.append(th)
                    nc.sync.dma_start(out=out[1, j * P:(j + 1) * P, mv],
                                      in_=t1s[bh][:])

                ot = gate_mm(2, j)
                for bh in range(2):
                    mv = slice(bh * NH, (bh + 1) * NH)
                    nc.vector.tensor_mul(t2s[bh][:], ot[:, mv], ths[bh][:])
                    nc.sync.dma_start(out=out[0, j * P:(j + 1) * P, mv],
                                      in_=t2s[bh][:])

    nc.finalize()
    return nc


def _pack_w(W, kdim):
    # pack[j, p, k*128+m] = W[j*128+m, k*128+p]
    kc = kdim // P
    return np.ascontiguousarray(
        W.reshape(JC, P, kc, P).transpose(0, 3, 2, 1).reshape(JC, P, kc * P))


def _prepare(inputs):
    f = lambda name: np.asarray(inputs[name], dtype=np.float32)

    gates = []
    for g in ("Wi", "Wf", "Wo", "Wc", "Ws"):
        u = "U" + g[1]
        gates.append(np.concatenate([f(g + "_w"), f(u + "_w")], axis=1))
    gates.append(f("a1_w"))
    wg6 = np.stack([_pack_w(w, K2) for w in gates])  # [6, JC, P, K2]

    wr = np.stack([_pack_w(f(n + "_w"), D) for n in ("r1", "r2", "r3")])

    a2p = np.ascontiguousarray(f("a2_w").reshape(KC1, P).T)  # [P, KC1]

    bias_vecs = []
    for g in ("Wi", "Wf", "Wo", "Wc", "Ws"):
        u = "U" + g[1]
        bias_vecs.append(f(g + "_b") + f(u + "_b"))
    bias_vecs += [f("a1_b"), f("r1_b"), f("r2_b"), f("r3_b"),
                  np.full(D, f("a2_b")[0], np.float32)]
    # biasp[p, v*JC + j] = vec_v[j*128 + p]
    biasp = np.ascontiguousarray(
        np.stack(bias_vecs).reshape(10, JC, P).transpose(2, 0, 1).reshape(P, 10 * JC))

    x, h, c = f("x"), f("h_prev"), f("c_prev")
    shared = {"wg6": wg6, "wr": wr, "a2p": a2p, "biasp": biasp}
    in_maps = []
    for core in range(NCORES):
        sl = slice(core * BL, (core + 1) * BL)
        xhT = np.ascontiguousarray(
            np.concatenate([x[sl].T, h[sl].T], axis=0))  # [K2, BL]
        cT = np.ascontiguousarray(c[sl].T)
        in_maps.append({**shared, "xhT": xhT, "cT": cT})
    return in_maps


def _run(inputs, trace=False):
    from concourse.bass_utils import run_bass_kernel_spmd

    if "nc" not in _CACHE:
        _CACHE["nc"] = _build()
    nc = _CACHE["nc"]
    in_maps = _prepare(inputs)
    res = run_bass_kernel_spmd(nc, in_maps, core_ids=list(range(NCORES)),
                               trace=trace)
    h = np.empty((B, D), np.float32)
    c = np.empty((B, D), np.float32)
    for core in range(NCORES):
        o = res.results[core]["out"]  # [2, D, BL]
        sl = slice(core * BL, (core + 1) * BL)
        h[sl] = o[0].T
        c[sl] = o[1].T
    return (h, c), res


def kernel(**inputs):
    (h, c), _ = _run(inputs, trace=False)
    return (h, c)



# revision 3
# speedup vs baseline: 1.4907x; 1.4907x over previous
"""AdaptiveLSTMCellWithRes on 8 TRN2 NeuronCores — mixed fp8/bf16.

Data-parallel over batch (1024 rows/core), weights replicated.
All on-chip compute happens in transposed-activation space [feat, batch].

Matmul precision (chosen so rel_err stays ~1.5e-2 < 2e-2 tolerance):
  - i, f, c_hat, s gates + alpha MLP (a1, a2): fp8 e4m3 with DoubleRow
    perf mode — two 128-deep k-tiles contracted per pass, 2x PE
    throughput. Weights pre-scaled x1024, activations x16, a1 stored
    x16; the scale is undone in the ScalarE activation that evicts PSUM.
  - o gate + residual chain r1/r2/r3: bf16 (their error feeds h_t/c_t
    directly, so fp8 would blow the tolerance).
Outputs h/c stream back as bf16 to halve the output DMA.
"""

import sys

if "/opt/trn_rl_repo" not in sys.path:
    sys.path.insert(0, "/opt/trn_rl_repo")

import numpy as np

P = 128
B = 8192          # global batch
NCORES = 8
BL = B // NCORES  # batch per core (1024)
D = 1024          # feature dim
K2 = 2048         # concat(x, h) contraction
JC = D // P       # 8 output-feature tiles
KC2 = K2 // P     # 16 k-chunks for gates/a1
KC1 = D // P      # 8 k-chunks for residual/a2
NQ2 = KC2 // 2    # 8 fp8 DoubleRow k-pairs for gates/a1
NH = BL // 2      # moving free dim per matmul (512)

AS = 16.0         # activation (x, h) fp8 scale
WS = 1024.0       # weight fp8 scale
RS = 16.0         # a1 relu-output fp8 scale

# fp8-packed gate order inside wg8
G8_I, G8_F, G8_C, G8_S, G8_A1 = 0, 1, 2, 3, 4

_CACHE = {}


def _build():
    import concourse.bass as bass  # noqa: F401
    from concourse import bacc, mybir
    import concourse.tile as tile

    F32 = mybir.dt.float32
    F8 = mybir.dt.float8e4
    BF = mybir.dt.bfloat16
    AF = mybir.ActivationFunctionType
    DR = mybir.MatmulPerfMode.DoubleRow

    nc = bacc.Bacc()

    # fp8 gate weights (i, f, c, s, a1): [5, JC, P, q, i, m] with
    # value = q8(W)[j*128+m, (2q+i)*128+p] * WS
    wg8 = nc.declare_dram_parameter("wg8", [5, JC, P, NQ2, 2, P], F8,
                                    isOutput=False)
    # o gate weights bf16: [JC, P, K2], pack[j, p, k*128+m] = W[j*128+m, k*128+p]
    wob = nc.declare_dram_parameter("wob", [JC, P, K2], BF, isOutput=False)
    # residual weights (r1, r2, r3) bf16: [3, JC, P, D]
    wr = nc.declare_dram_parameter("wr", [3, JC, P, D], BF, isOutput=False)
    # a2 weight fp8: [P, KC1] with a2p[p, k] = q8(a2_w)[0, k*128+p] * WS
    a2p = nc.declare_dram_parameter("a2p", [P, KC1], F8, isOutput=False)
    # biases: [P, 10*JC]; col v*JC+j holds vec_v[j*128:(j+1)*128]
    # v: 0..4 = combined gate biases (i,f,o,c,s), 5=a1_b*RS, 6=r1_b,
    # 7=r2_b, 8=r3_b, 9=a2_b (replicated)
    biasp = nc.declare_dram_parameter("biasp", [P, 10 * JC], F32, isOutput=False)
    # fp8 transposed activations packed for DoubleRow:
    # xh8[q, p, i, n] = q8(concat(x,h)^T * AS)[(2q+i)*128+p, n]
    xh8 = nc.declare_dram_parameter("xh8", [NQ2, P, 2, BL], F8, isOutput=False)
    # bf16 transposed activations (o gate needs all; r1 needs rows D..2D-1)
    xhb = nc.declare_dram_parameter("xhb", [K2, BL], BF, isOutput=False)
    cT = nc.declare_dram_parameter("cT", [D, BL], F32, isOutput=False)
    # out[0] = h_t^T, out[1] = c_t^T (bf16)
    out = nc.declare_dram_parameter("out", [2, D, BL], BF, isOutput=True)

    alpha_dram = nc.dram_tensor("alpha_dram", [1, BL], F32)

    GSC = 1.0 / (AS * WS)   # gate PSUM descale
    A1SC = RS / (AS * WS)   # a1 PSUM scale (stores a1*RS)
    A2SC = 1.0 / (RS * WS)  # a2 PSUM descale

    with tile.TileContext(nc) as tc:
        with (
            tc.tile_pool(name="consts", bufs=1) as consts,
            tc.tile_pool(name="xh", bufs=1) as xh_pool,
            tc.tile_pool(name="w", bufs=6) as w_pool,
            tc.tile_pool(name="a1s", bufs=4) as a1_pool,
            tc.tile_pool(name="r1", bufs=1) as r1_pool,
            tc.tile_pool(name="r2", bufs=1) as r2_pool,
            tc.tile_pool(name="gates", bufs=1) as g_pool,
            tc.tile_pool(name="ew", bufs=2) as ew_pool,
            tc.tile_pool(name="psum", bufs=3, space="PSUM") as psum_pool,
            tc.tile_pool(name="psum_a2", bufs=1, space="PSUM") as psum_a2_pool,
        ):
            bias_sb = consts.tile([P, 10 * JC], F32, name="bias_sb")
            a2_sb = consts.tile([P, KC1], F8, name="a2_sb")

            def bias_ap(v, j):
                return bias_sb[:, v * JC + j: v * JC + j + 1]

            # bf16 xh tiles, one per k-chunk; h-half (k 8..15) DMAs split
            # per batch-half so r1's first matmuls start sooner
            xhbt = [None] * KC2

            def load_xhb(k, split):
                t = xh_pool.tile([P, BL], BF, tag=f"xhb{k}", name=f"xhb{k}")
                if split:
                    for bh in range(2):
                        nc.sync.dma_start(
                            out=t[:, bh * NH:(bh + 1) * NH],
                            in_=xhb[k * P:(k + 1) * P, bh * NH:(bh + 1) * NH])
                else:
                    nc.sync.dma_start(out=t[:], in_=xhb[k * P:(k + 1) * P, :])
                xhbt[k] = t

            # fp8 DoubleRow xh pair tiles, one per k-pair
            xh8t = [None] * NQ2

            def load_xh8(q):
                t = xh_pool.tile([P, 2, BL], F8, tag=f"xh8{q}", name=f"xh8{q}")
                nc.sync.dma_start(out=t[:], in_=xh8[q])
                xh8t[q] = t

            def load_w8(g, j):
                # one fp8 slab covers the whole K2 contraction (2KB/partition)
                wt = w_pool.tile([P, NQ2, 2, P], F8, tag="w", name=f"w8_{g}_{j}")
                nc.sync.dma_start(out=wt[:], in_=wg8[g, j])
                return wt

            def load_wb(src_ap2, nslabs, name):
                # bf16 slabs of [P, D] (2KB/partition)
                slabs = []
                for i in range(nslabs):
                    wt = w_pool.tile([P, D], BF, tag="w", name=f"{name}{i}")
                    nc.sync.dma_start(out=wt[:], in_=src_ap2[:, i * D:(i + 1) * D])
                    slabs.append(wt)
                return slabs

            def mm8(ps2, wt, rhs_pairs):
                # fp8 DoubleRow: 2 k-tiles per pass over NQ2 pairs
                for q in range(NQ2):
                    for bh in range(2):
                        mv = slice(bh * NH, (bh + 1) * NH)
                        nc.tensor.matmul(
                            ps2[bh][:], wt[:, q], rhs_pairs[q][:, :, mv],
                            start=(q == 0), stop=(q == NQ2 - 1), perf_mode=DR)

            def mmb(ps2, wslabs, rhs_tiles, kc):
                # bf16: k outer / bh inner; KC1 k-chunks per slab
                for k in range(kc):
                    wt = wslabs[k // KC1]
                    kk = k % KC1
                    for bh in range(2):
                        mv = slice(bh * NH, (bh + 1) * NH)
                        nc.tensor.matmul(
                            ps2[bh][:], wt[:, kk * P:(kk + 1) * P],
                            rhs_tiles[k][:, mv],
                            start=(k == 0), stop=(k == kc - 1))

            # first two r1 weight slabs must beat the xh stream so the PE
            # can start as soon as the first h tiles land
            r1w_pre = [load_wb(wr[0, 0], 1, "wt_r1p0"),
                       load_wb(wr[0, 1], 1, "wt_r1p1")]
            for k in range(KC1, KC2):
                load_xhb(k, split=True)
            for q in range(NQ2):
                load_xh8(q)
            # consts aren't needed until the first PSUM eviction;
            # keep them out of the DMA queues' critical prefix
            nc.sync.dma_start(out=bias_sb[:], in_=biasp[:, :])
            nc.sync.dma_start(out=a2_sb[:], in_=a2p[:, :])

            # ---- phase A: r1 (bf16, h-half of xhb); a1 -> a2 (fp8); r2 ----
            r1 = []
            for j in range(JC):
                ws = r1w_pre[j] if j < 2 else load_wb(wr[0, j], 1, "wt_r1")
                t = r1_pool.tile([P, BL], BF, tag=f"r1_{j}", name=f"r1_{j}")
                ps2 = [psum_pool.tile([P, NH], F32, tag="ps0", name="ps_r1_0"),
                       psum_pool.tile([P, NH], F32, tag="ps1", name="ps_r1_1")]
                mmb(ps2, ws, xhbt[KC1:], KC1)
                for bh in range(2):
                    nc.scalar.activation(t[:, bh * NH:(bh + 1) * NH], ps2[bh][:],
                                         AF.Relu, bias=bias_ap(6, j))
                r1.append(t)

            # x-half bf16 loads (o gate, phase B) overlap r1 compute
            for k in range(KC1):
                load_xhb(k, split=False)

            ps_a2 = [psum_a2_pool.tile([1, NH], F32, tag="a20", name="psa20"),
                     psum_a2_pool.tile([1, NH], F32, tag="a21", name="psa21")]
            pend = []

            def flush_a2():
                jq, pair = pend.pop(0)
                for bh in range(2):
                    nc.tensor.matmul(ps_a2[bh][:], a2_sb[:, jq:jq + 1],
                                     pair[bh][:], start=(jq == 0),
                                     stop=(jq == JC - 1))

            for j in range(JC):
                wt = load_w8(G8_A1, j)
                ps2 = [psum_pool.tile([P, NH], F32, tag="ps0", name="ps_a1_0"),
                       psum_pool.tile([P, NH], F32, tag="ps1", name="ps_a1_1")]
                mm8(ps2, wt, xh8t)
                pair = []
                for bh in range(2):
                    a1b = a1_pool.tile([P, NH], F8, tag="a1", name="a1b")
                    nc.scalar.activation(a1b[:], ps2[bh][:], AF.Relu,
                                         bias=bias_ap(5, j), scale=A1SC)
                    pair.append(a1b)
                pend.append((j, pair))
                # defer the tiny a2 matmuls one j so PE never waits on ScalarE
                if len(pend) == 2:
                    flush_a2()
            while pend:
                flush_a2()

            r2 = []
            for j in range(JC):
                ws = load_wb(wr[1, j], 1, "wt_r2")
                t = r2_pool.tile([P, BL], BF, tag=f"r2_{j}", name=f"r2_{j}")
                ps2 = [psum_pool.tile([P, NH], F32, tag="ps0", name="ps_r2_0"),
                       psum_pool.tile([P, NH], F32, tag="ps1", name="ps_r2_1")]
                mmb(ps2, ws, r1, KC1)
                for bh in range(2):
                    nc.scalar.activation(t[:, bh * NH:(bh + 1) * NH], ps2[bh][:],
                                         AF.Relu, bias=bias_ap(7, j))
                r2.append(t)

            # alpha = sigmoid(a2 @ a1relu + a2_b): [1, BL]; broadcast via DRAM
            for bh in range(2):
                asb = a1_pool.tile([1, NH], F32, tag="a1", name="alpha_sb")
                nc.scalar.activation(asb[:], ps_a2[bh][:], AF.Sigmoid,
                                     bias=bias_sb[0:1, 9 * JC: 9 * JC + 1],
                                     scale=A2SC)
                nc.sync.dma_start(out=alpha_dram[0:1, bh * NH:(bh + 1) * NH],
                                  in_=asb[:])
            alpha_rep = consts.tile([P, BL], F32, name="alpha_rep")
            nc.gpsimd.dma_start(
                out=alpha_rep[:], in_=alpha_dram[0:1, :].broadcast_to([P, BL]))

            # ---- phase B: gates + r3 + combine, per feature tile j.
            # Gate order c,s,i,f,r3,o lets the elementwise chain run while
            # later matmuls stream, so only h=o*tanh(c) trails the last MM.
            def gate8(g, j, fn, v):
                wt = load_w8(g, j)
                t = g_pool.tile([P, BL], F32, tag=f"g{g}", name=f"g{g}")
                ps2 = [psum_pool.tile([P, NH], F32, tag="ps0", name="ps_g0"),
                       psum_pool.tile([P, NH], F32, tag="ps1", name="ps_g1")]
                mm8(ps2, wt, xh8t)
                for bh in range(2):
                    nc.scalar.activation(t[:, bh * NH:(bh + 1) * NH],
                                         ps2[bh][:], fn,
                                         bias=bias_ap(v, j), scale=GSC)
                return t

            for j in range(JC):
                ch = gate8(G8_C, j, AF.Tanh, 3)
                st = gate8(G8_S, j, AF.Sigmoid, 4)
                it = gate8(G8_I, j, AF.Sigmoid, 0)

                t1s, t2s, ths = [], [], []
                for bh in range(2):
                    mv = slice(bh * NH, (bh + 1) * NH)
                    t1 = ew_pool.tile([P, NH], F32, tag=f"t1{bh}", name="t1")
                    nc.vector.tensor_mul(t1[:], it[:, mv], ch[:, mv])
                    nc.vector.tensor_mul(t1[:], t1[:], st[:, mv])
                    nc.vector.tensor_mul(t1[:], t1[:], alpha_rep[:, mv])
                    t1s.append(t1)

                ft = gate8(G8_F, j, AF.Sigmoid, 1)
                for bh in range(2):
                    mv = slice(bh * NH, (bh + 1) * NH)
                    cp = ew_pool.tile([P, NH], F32, tag="cp", name="cp", bufs=1)
                    nc.sync.dma_start(out=cp[:], in_=cT[j * P:(j + 1) * P, mv])
                    t2 = ew_pool.tile([P, NH], F32, tag=f"t2{bh}", name="t2")
                    nc.vector.tensor_mul(t2[:], ft[:, mv], cp[:])
                    nc.vector.tensor_add(t1s[bh][:], t1s[bh][:], t2[:])
                    t2s.append(t2)

                ws = load_wb(wr[2, j], 1, "wt_r3")
                r3 = g_pool.tile([P, BL], F32, tag="r3", name="r3")
                ps2 = [psum_pool.tile([P, NH], F32, tag="ps0", name="ps_r3_0"),
                       psum_pool.tile([P, NH], F32, tag="ps1", name="ps_r3_1")]
                mmb(ps2, ws, r2, KC1)
                for bh in range(2):
                    nc.scalar.activation(r3[:, bh * NH:(bh + 1) * NH], ps2[bh][:],
                                         AF.Identity, bias=bias_ap(8, j))
                for bh in range(2):
                    mv = slice(bh * NH, (bh + 1) * NH)
                    cb = ew_pool.tile([P, NH], BF, tag=f"cb{bh}", name="cb",
                                      bufs=1)
                    nc.vector.tensor_add(cb[:], t1s[bh][:], r3[:, mv])
                    nc.sync.dma_start(out=out[1, j * P:(j + 1) * P, mv],
                                      in_=cb[:])
                    th = ew_pool.tile([P, NH], F32, tag=f"th{bh}", name="th",
                                      bufs=1)
                    nc.scalar.activation(th[:], cb[:], AF.Tanh)
                    ths.append(th)

                ws_o = load_wb(wob[j], 2, "wt_o")
                ot = g_pool.tile([P, BL], F32, tag="go", name="go")
                ps2 = [psum_pool.tile([P, NH], F32, tag="ps0", name="ps_o0"),
                       psum_pool.tile([P, NH], F32, tag="ps1", name="ps_o1")]
                mmb(ps2, ws_o, xhbt, KC2)
                for bh in range(2):
                    nc.scalar.activation(ot[:, bh * NH:(bh + 1) * NH],
                                         ps2[bh][:], AF.Sigmoid,
                                         bias=bias_ap(2, j))
                for bh in range(2):
                    mv = slice(bh * NH, (bh + 1) * NH)
                    hb = ew_pool.tile([P, NH], BF, tag=f"hb{bh}", name="hb",
                                      bufs=1)
                    nc.vector.tensor_mul(hb[:], ot[:, mv], ths[bh][:])
                    nc.sync.dma_start(out=out[0, j * P:(j + 1) * P, mv],
                                      in_=hb[:])

    nc.finalize()
    return nc


def _pack_w(W, kdim):
    # pack[j, p, k*128+m] = W[j*128+m, k*128+p]
    kc = kdim // P
    return np.ascontiguousarray(
        W.reshape(JC, P, kc, P).transpose(0, 3, 2, 1).reshape(JC, P, kc * P))


def _prepare(inputs):
    import ml_dtypes
    F8NP = ml_dtypes.float8_e4m3
    BF16 = ml_dtypes.bfloat16

    f = lambda name: np.asarray(inputs[name], dtype=np.float32)

    def q8(a):
        return np.clip(a * WS, -240, 240).astype(F8NP)

    # fp8 gates: i, f, c, s (W|U fused), a1
    g8 = []
    for g in ("Wi", "Wf", "Wc", "Ws"):
        u = "U" + g[1]
        g8.append(np.concatenate([f(g + "_w"), f(u + "_w")], axis=1))
    g8.append(f("a1_w"))
    # [5, JC, P, K2] -> [5, JC, P, NQ2, 2, P] (k-major order means adjacent
    # k-chunks are already adjacent in the packed free dim)
    wg8 = np.stack([q8(_pack_w(w, K2)) for w in g8]).reshape(
        5, JC, P, NQ2, 2, P)

    wob = np.ascontiguousarray(_pack_w(
        np.concatenate([f("Wo_w"), f("Uo_w")], axis=1), K2).astype(BF16))
    wr = np.stack([_pack_w(f(n + "_w"), D) for n in ("r1", "r2", "r3")]
                  ).astype(BF16)
    a2p = q8(np.ascontiguousarray(f("a2_w").reshape(KC1, P).T))  # [P, KC1]

    bias_vecs = []
    for g in ("Wi", "Wf", "Wo", "Wc", "Ws"):
        u = "U" + g[1]
        bias_vecs.append(f(g + "_b") + f(u + "_b"))
    bias_vecs += [f("a1_b") * RS, f("r1_b"), f("r2_b"), f("r3_b"),
                  np.full(D, f("a2_b")[0], np.float32)]
    # biasp[p, v*JC + j] = vec_v[j*128 + p]
    biasp = np.ascontiguousarray(
        np.stack(bias_vecs).reshape(10, JC, P).transpose(2, 0, 1).reshape(
            P, 10 * JC))

    x, h, c = f("x"), f("h_prev"), f("c_prev")
    shared = {"wg8": wg8, "wob": wob, "wr": wr, "a2p": a2p, "biasp": biasp}
    in_maps = []
    for core in range(NCORES):
        sl = slice(core * BL, (core + 1) * BL)
        xhT = np.ascontiguousarray(
            np.concatenate([x[sl].T, h[sl].T], axis=0))  # [K2, BL]
        xh8 = np.ascontiguousarray(
            np.clip(xhT * AS, -240, 240).astype(F8NP).reshape(
                NQ2, 2, P, BL).transpose(0, 2, 1, 3))  # [NQ2, P, 2, BL]
        xhb = xhT.astype(BF16)
        cTc = np.ascontiguousarray(c[sl].T)
        in_maps.append({**shared, "xh8": xh8, "xhb": xhb, "cT": cTc})
    return in_maps


def _run(inputs, trace=False):
    from concourse.bass_utils import run_bass_kernel_spmd

    if "nc" not in _CACHE:
        _CACHE["nc"] = _build()
    nc = _CACHE["nc"]
    in_maps = _prepare(inputs)
    res = run_bass_kernel_spmd(nc, in_maps, core_ids=list(range(NCORES)),
                               trace=trace)
    h = np.empty((B, D), np.float32)
    c = np.empty((B, D), np.float32)
    for core in range(NCORES):
        o = res.results[core]["out"]  # [2, D, BL] bf16
        sl = slice(core * BL, (core + 1) * BL)
        h[sl] = o[0].T.astype(np.float32)
        c[sl] = o[1].T.astype(np.float32)
    return (h, c), res


def kernel(**inputs):
    (h, c), _ = _run(inputs, trace=False)
    return (h, c)


# revision 5
# speedup vs baseline: 1.5358x; 1.0302x over previous
"""AdaptiveLSTMCellWithRes on 8 TRN2 NeuronCores — mixed fp8/bf16.

Data-parallel over batch (1024 rows/core), weights replicated.
All on-chip compute happens in transposed-activation space [feat, batch].

Matmul precision (chosen so rel_err stays ~1.6e-2 < 2e-2 tolerance):
  - i, f, c_hat, s gates + alpha MLP (a1, a2) + o gate x-half: fp8 e4m3
    with DoubleRow perf mode — two 128-deep k-tiles contracted per pass,
    2x PE throughput. Weights pre-scaled x1024, activations x16, a1
    stored x16; the scale is undone in the ScalarE activation that
    evicts PSUM.
  - o gate h-half + residual chain r1/r2/r3: bf16 (their error feeds
    h_t/c_t directly, so full fp8 would blow the tolerance). The o
    gate's bf16 half shares a PSUM group with its fp8 half; its Uo
    weights are pre-scaled by AS*WS (exact power of 2) so both halves
    carry the same scale.
Outputs h/c stream back as bf16 to halve the output DMA.
"""

import sys

if "/opt/trn_rl_repo" not in sys.path:
    sys.path.insert(0, "/opt/trn_rl_repo")

import numpy as np

P = 128
B = 8192          # global batch
NCORES = 8
BL = B // NCORES  # batch per core (1024)
D = 1024          # feature dim
K2 = 2048         # concat(x, h) contraction
JC = D // P       # 8 output-feature tiles
KC2 = K2 // P     # 16 k-chunks for gates/a1
KC1 = D // P      # 8 k-chunks for residual/a2/o-halves
NQ2 = KC2 // 2    # 8 fp8 DoubleRow k-pairs for gates/a1
NQ1 = KC1 // 2    # 4 k-pairs for the o gate's x-half
NH = BL // 2      # moving free dim per matmul (512)

AS = 16.0         # activation (x, h) fp8 scale
WS = 1024.0       # weight fp8 scale
RS = 16.0         # a1 relu-output fp8 scale

# fp8-packed gate order inside wg8
G8_I, G8_F, G8_C, G8_S, G8_A1 = 0, 1, 2, 3, 4

_CACHE = {}


def _build():
    import concourse.bass as bass  # noqa: F401
    from concourse import bacc, mybir
    import concourse.tile as tile

    F32 = mybir.dt.float32
    F8 = mybir.dt.float8e4
    BF = mybir.dt.bfloat16
    AF = mybir.ActivationFunctionType
    DR = mybir.MatmulPerfMode.DoubleRow

    nc = bacc.Bacc()

    # fp8 gate weights (i, f, c, s, a1): [5, JC, P, q, i, m] with
    # value = q8(W)[j*128+m, (2q+i)*128+p] * WS
    wg8 = nc.declare_dram_parameter("wg8", [5, JC, P, NQ2, 2, P], F8,
                                    isOutput=False)
    # o gate x-half fp8 (scaled by WS/AS-consistent GSC trick below is not
    # possible in a mixed PSUM group, so the h-half weights are pre-scaled
    # UP by AS*WS instead — see _prepare: wou holds Uo * AS*WS in bf16,
    # giving both halves the same AS*WS PSUM scale)
    wox8 = nc.declare_dram_parameter("wox8", [JC, P, NQ1, 2, P], F8,
                                     isOutput=False)
    wou = nc.declare_dram_parameter("wou", [JC, P, D], BF, isOutput=False)
    # residual weights (r1, r2, r3) bf16: [3, JC, P, D]
    wr = nc.declare_dram_parameter("wr", [3, JC, P, D], BF, isOutput=False)
    # a2 weight fp8: [P, KC1] with a2p[p, k] = q8(a2_w)[0, k*128+p] * WS
    a2p = nc.declare_dram_parameter("a2p", [P, KC1], F8, isOutput=False)
    # biases: [P, 10*JC]; col v*JC+j holds vec_v[j*128:(j+1)*128]
    # v: 0..4 = combined gate biases (i,f,o,c,s), 5=a1_b*RS, 6=r1_b,
    # 7=r2_b, 8=r3_b, 9=a2_b (replicated)
    biasp = nc.declare_dram_parameter("biasp", [P, 10 * JC], F32, isOutput=False)
    # fp8 transposed activations packed for DoubleRow:
    # xh8[q, p, i, n] = q8(concat(x,h)^T * AS)[(2q+i)*128+p, n]
    xh8 = nc.declare_dram_parameter("xh8", [NQ2, P, 2, BL], F8, isOutput=False)
    # bf16 transposed h (o gate h-half and r1)
    hTb = nc.declare_dram_parameter("hTb", [D, BL], BF, isOutput=False)
    cT = nc.declare_dram_parameter("cT", [D, BL], F32, isOutput=False)
    # out[0] = h_t^T, out[1] = c_t^T (bf16)
    out = nc.declare_dram_parameter("out", [2, D, BL], BF, isOutput=True)

    alpha_dram = nc.dram_tensor("alpha_dram", [1, BL], F32)

    GSC = 1.0 / (AS * WS)   # gate PSUM descale
    A1SC = RS / (AS * WS)   # a1 PSUM scale (stores a1*RS)
    A2SC = 1.0 / (RS * WS)  # a2 PSUM descale

    with tile.TileContext(nc) as tc:
        with (
            tc.tile_pool(name="consts", bufs=1) as consts,
            tc.tile_pool(name="xh", bufs=1) as xh_pool,
            tc.tile_pool(name="w", bufs=6) as w_pool,
            tc.tile_pool(name="a1s", bufs=4) as a1_pool,
            tc.tile_pool(name="r1", bufs=1) as r1_pool,
            tc.tile_pool(name="r2", bufs=1) as r2_pool,
            tc.tile_pool(name="gates", bufs=1) as g_pool,
            tc.tile_pool(name="ew", bufs=2) as ew_pool,
            tc.tile_pool(name="psum", bufs=3, space="PSUM") as psum_pool,
            tc.tile_pool(name="psum_a2", bufs=1, space="PSUM") as psum_a2_pool,
        ):
            bias_sb = consts.tile([P, 10 * JC], F32, name="bias_sb")
            a2_sb = consts.tile([P, KC1], F8, name="a2_sb")

            def bias_ap(v, j):
                return bias_sb[:, v * JC + j: v * JC + j + 1]

            # bf16 h tiles, one per k-chunk of h^T (k = 0..7); DMAs split
            # per batch-half, all bh0 halves first, so r1's bh-outer
            # matmul order can start as soon as the first halves land
            hbt = [None] * KC1

            def alloc_hbt():
                for k in range(KC1):
                    hbt[k] = xh_pool.tile([P, BL], BF, tag=f"hb{k}",
                                          name=f"hb{k}")

            def load_hbt_half(k, bh):
                nc.sync.dma_start(
                    out=hbt[k][:, bh * NH:(bh + 1) * NH],
                    in_=hTb[k * P:(k + 1) * P, bh * NH:(bh + 1) * NH])

            # fp8 DoubleRow xh pair tiles, one per k-pair
            xh8t = [None] * NQ2

            def load_xh8(q):
                t = xh_pool.tile([P, 2, BL], F8, tag=f"xh8{q}", name=f"xh8{q}")
                nc.sync.dma_start(out=t[:], in_=xh8[q])
                xh8t[q] = t

            def load_w8(g, j):
                # one fp8 slab covers the whole K2 contraction (2KB/partition)
                wt = w_pool.tile([P, NQ2, 2, P], F8, tag="w", name=f"w8_{g}_{j}")
                nc.sync.dma_start(out=wt[:], in_=wg8[g, j])
                return wt

            def load_wb(src_ap2, nchunks, name):
                # bf16 [P, D] slab in `nchunks` DMA chunks along k
                kpc = KC1 // nchunks
                slabs = []
                for i in range(nchunks):
                    wt = w_pool.tile([P, kpc * P], BF, tag="w",
                                     name=f"{name}{i}")
                    nc.sync.dma_start(
                        out=wt[:], in_=src_ap2[:, i * kpc * P:(i + 1) * kpc * P])
                    slabs.append(wt)
                return slabs, kpc

            def mm8(ps2, wt, rhs_pairs):
                # fp8 DoubleRow: 2 k-tiles per pass; bh outer so ScalarE
                # can evict bh0 while bh1 streams
                for bh in range(2):
                    mv = slice(bh * NH, (bh + 1) * NH)
                    for q in range(NQ2):
                        nc.tensor.matmul(
                            ps2[bh][:], wt[:, q], rhs_pairs[q][:, :, mv],
                            start=(q == 0), stop=(q == NQ2 - 1), perf_mode=DR)

            def mmb(ps2, wslabs, kpc, rhs_tiles, kc):
                # bf16: bh outer / k inner
                for bh in range(2):
                    mv = slice(bh * NH, (bh + 1) * NH)
                    for k in range(kc):
                        wt = wslabs[k // kpc]
                        kk = k % kpc
                        nc.tensor.matmul(
                            ps2[bh][:], wt[:, kk * P:(kk + 1) * P],
                            rhs_tiles[k][:, mv],
                            start=(k == 0), stop=(k == kc - 1))

            # ---- DMA critical prefix: r1 j0/j1 slabs (chunked), then the
            # h tiles r1 consumes, then the remaining r1 slabs, then the
            # fp8 xh stream (first needed by a1, ~40us in)
            alloc_hbt()
            r1w_pre = [load_wb(wr[0, 0], 2, "wt_r1p0"),
                       load_wb(wr[0, 1], 2, "wt_r1p1")]
            for bh in range(2):
                for k in range(KC1):
                    load_hbt_half(k, bh)
            r1w_rest = [load_wb(wr[0, j], 1, "wt_r1") for j in range(2, JC)]
            for q in range(NQ2):
                load_xh8(q)
            # consts aren't needed until the first PSUM eviction;
            # keep them out of the DMA queues' critical prefix
            nc.sync.dma_start(out=bias_sb[:], in_=biasp[:, :])
            nc.sync.dma_start(out=a2_sb[:], in_=a2p[:, :])

            # ---- phase A: r1 (bf16, h); a1 -> a2 (fp8); r2 ----
            r1 = []
            for j in range(JC):
                ws, kpc = r1w_pre[j] if j < 2 else r1w_rest[j - 2]
                t = r1_pool.tile([P, BL], BF, tag=f"r1_{j}", name=f"r1_{j}")
                ps2 = [psum_pool.tile([P, NH], F32, tag="ps0", name="ps_r1_0"),
                       psum_pool.tile([P, NH], F32, tag="ps1", name="ps_r1_1")]
                mmb(ps2, ws, kpc, hbt, KC1)
                for bh in range(2):
                    nc.scalar.activation(t[:, bh * NH:(bh + 1) * NH], ps2[bh][:],
                                         AF.Relu, bias=bias_ap(6, j))
                r1.append(t)

            ps_a2 = [psum_a2_pool.tile([1, NH], F32, tag="a20", name="psa20"),
                     psum_a2_pool.tile([1, NH], F32, tag="a21", name="psa21")]
            pend = []

            def flush_a2():
                jq, pair = pend.pop(0)
                for bh in range(2):
                    nc.tensor.matmul(ps_a2[bh][:], a2_sb[:, jq:jq + 1],
                                     pair[bh][:], start=(jq == 0),
                                     stop=(jq == JC - 1))

            for j in range(JC):
                wt = load_w8(G8_A1, j)
                ps2 = [psum_pool.tile([P, NH], F32, tag="ps0", name="ps_a1_0"),
                       psum_pool.tile([P, NH], F32, tag="ps1", name="ps_a1_1")]
                mm8(ps2, wt, xh8t)
                pair = []
                for bh in range(2):
                    a1b = a1_pool.tile([P, NH], F8, tag="a1", name="a1b")
                    nc.scalar.activation(a1b[:], ps2[bh][:], AF.Relu,
                                         bias=bias_ap(5, j), scale=A1SC)
                    pair.append(a1b)
                pend.append((j, pair))
                # defer the tiny a2 matmuls one j so PE never waits on ScalarE
                if len(pend) == 2:
                    flush_a2()
            while pend:
                flush_a2()

            r2 = []
            for j in range(JC):
                ws, kpc = load_wb(wr[1, j], 1, "wt_r2")
                t = r2_pool.tile([P, BL], BF, tag=f"r2_{j}", name=f"r2_{j}")
                ps2 = [psum_pool.tile([P, NH], F32, tag="ps0", name="ps_r2_0"),
                       psum_pool.tile([P, NH], F32, tag="ps1", name="ps_r2_1")]
                mmb(ps2, ws, kpc, r1, KC1)
                for bh in range(2):
                    nc.scalar.activation(t[:, bh * NH:(bh + 1) * NH], ps2[bh][:],
                                         AF.Relu, bias=bias_ap(7, j))
                r2.append(t)

            # alpha = sigmoid(a2 @ a1relu + a2_b): [1, BL]; broadcast via DRAM
            for bh in range(2):
                asb = a1_pool.tile([1, NH], F32, tag="a1", name="alpha_sb")
                nc.scalar.activation(asb[:], ps_a2[bh][:], AF.Sigmoid,
                                     bias=bias_sb[0:1, 9 * JC: 9 * JC + 1],
                                     scale=A2SC)
                nc.sync.dma_start(out=alpha_dram[0:1, bh * NH:(bh + 1) * NH],
                                  in_=asb[:])
            alpha_rep = consts.tile([P, BL], F32, name="alpha_rep")
            nc.gpsimd.dma_start(
                out=alpha_rep[:], in_=alpha_dram[0:1, :].broadcast_to([P, BL]))

            # ---- phase B: gates + r3 + combine, per feature tile j.
            # Gate order c,s,i,f,r3,o lets the elementwise chain run while
            # later matmuls stream, so only h=o*tanh(c) trails the last MM.
            def gate8(g, j, fn, v):
                wt = load_w8(g, j)
                t = g_pool.tile([P, BL], F32, tag=f"g{g}", name=f"g{g}")
                ps2 = [psum_pool.tile([P, NH], F32, tag="ps0", name="ps_g0"),
                       psum_pool.tile([P, NH], F32, tag="ps1", name="ps_g1")]
                mm8(ps2, wt, xh8t)
                for bh in range(2):
                    nc.scalar.activation(t[:, bh * NH:(bh + 1) * NH],
                                         ps2[bh][:], fn,
                                         bias=bias_ap(v, j), scale=GSC)
                return t

            for j in range(JC):
                ch = gate8(G8_C, j, AF.Tanh, 3)
                st = gate8(G8_S, j, AF.Sigmoid, 4)
                it = gate8(G8_I, j, AF.Sigmoid, 0)

                t1s, t2s, ths = [], [], []
                for bh in range(2):
                    mv = slice(bh * NH, (bh + 1) * NH)
                    t1 = ew_pool.tile([P, NH], F32, tag=f"t1{bh}", name="t1")
                    nc.vector.tensor_mul(t1[:], it[:, mv], ch[:, mv])
                    nc.vector.tensor_mul(t1[:], t1[:], st[:, mv])
                    nc.vector.tensor_mul(t1[:], t1[:], alpha_rep[:, mv])
                    t1s.append(t1)

                ft = gate8(G8_F, j, AF.Sigmoid, 1)
                for bh in range(2):
                    mv = slice(bh * NH, (bh + 1) * NH)
                    cp = ew_pool.tile([P, NH], F32, tag="cp", name="cp", bufs=1)
                    nc.sync.dma_start(out=cp[:], in_=cT[j * P:(j + 1) * P, mv])
                    t2 = ew_pool.tile([P, NH], F32, tag=f"t2{bh}", name="t2")
                    nc.vector.tensor_mul(t2[:], ft[:, mv], cp[:])
                    nc.vector.tensor_add(t1s[bh][:], t1s[bh][:], t2[:])
                    t2s.append(t2)

                ws, kpc = load_wb(wr[2, j], 1, "wt_r3")
                r3 = g_pool.tile([P, BL], F32, tag="r3", name="r3")
                ps2 = [psum_pool.tile([P, NH], F32, tag="ps0", name="ps_r3_0"),
                       psum_pool.tile([P, NH], F32, tag="ps1", name="ps_r3_1")]
                mmb(ps2, ws, kpc, r2, KC1)
                for bh in range(2):
                    nc.scalar.activation(r3[:, bh * NH:(bh + 1) * NH], ps2[bh][:],
                                         AF.Identity, bias=bias_ap(8, j))
                for bh in range(2):
                    mv = slice(bh * NH, (bh + 1) * NH)
                    cb = ew_pool.tile([P, NH], BF, tag=f"cb{bh}", name="cb",
                                      bufs=1)
                    nc.vector.tensor_add(cb[:], t1s[bh][:], r3[:, mv])
                    nc.sync.dma_start(out=out[1, j * P:(j + 1) * P, mv],
                                      in_=cb[:])
                    th = ew_pool.tile([P, NH], F32, tag=f"th{bh}", name="th",
                                      bufs=1)
                    nc.scalar.activation(th[:], cb[:], AF.Tanh)
                    ths.append(th)

                # o gate: x-half fp8 DoubleRow + h-half bf16 share one PSUM
                # group (wou is pre-scaled by AS*WS so scales match)
                wox = w_pool.tile([P, NQ1, 2, P], F8, tag="w", name="wox")
                nc.sync.dma_start(out=wox[:], in_=wox8[j])
                wsu, kpcu = load_wb(wou[j], 1, "wt_ou")
                ot = g_pool.tile([P, BL], F32, tag="go", name="go")
                ps2 = [psum_pool.tile([P, NH], F32, tag="ps0", name="ps_o0"),
                       psum_pool.tile([P, NH], F32, tag="ps1", name="ps_o1")]
                for bh in range(2):
                    mv = slice(bh * NH, (bh + 1) * NH)
                    for q in range(NQ1):
                        nc.tensor.matmul(
                            ps2[bh][:], wox[:, q], xh8t[q][:, :, mv],
                            start=(q == 0), stop=False, perf_mode=DR)
                    for k in range(KC1):
                        nc.tensor.matmul(
                            ps2[bh][:], wsu[0][:, k * P:(k + 1) * P],
                            hbt[k][:, mv], start=False, stop=(k == KC1 - 1))
                for bh in range(2):
                    nc.scalar.activation(ot[:, bh * NH:(bh + 1) * NH],
                                         ps2[bh][:], AF.Sigmoid,
                                         bias=bias_ap(2, j), scale=GSC)
                for bh in range(2):
                    mv = slice(bh * NH, (bh + 1) * NH)
                    hb = ew_pool.tile([P, NH], BF, tag=f"hb{bh}", name="hb",
                                      bufs=1)
                    nc.vector.tensor_mul(hb[:], ot[:, mv], ths[bh][:])
                    nc.sync.dma_start(out=out[0, j * P:(j + 1) * P, mv],
                                      in_=hb[:])

    nc.finalize()
    return nc


def _pack_w(W, kdim):
    # pack[j, p, k*128+m] = W[j*128+m, k*128+p]
    kc = kdim // P
    return np.ascontiguousarray(
        W.reshape(JC, P, kc, P).transpose(0, 3, 2, 1).reshape(JC, P, kc * P))


def _prepare(inputs):
    import ml_dtypes
    F8NP = ml_dtypes.float8_e4m3
    BF16 = ml_dtypes.bfloat16

    f = lambda name: np.asarray(inputs[name], dtype=np.float32)

    def q8(a):
        return np.clip(a * WS, -240, 240).astype(F8NP)

    # fp8 gates: i, f, c, s (W|U fused), a1
    g8 = []
    for g in ("Wi", "Wf", "Wc", "Ws"):
        u = "U" + g[1]
        g8.append(np.concatenate([f(g + "_w"), f(u + "_w")], axis=1))
    g8.append(f("a1_w"))
    # [5, JC, P, K2] -> [5, JC, P, NQ2, 2, P] (k-major order means adjacent
    # k-chunks are already adjacent in the packed free dim)
    wg8 = np.stack([q8(_pack_w(w, K2)) for w in g8]).reshape(
        5, JC, P, NQ2, 2, P)

    # o gate: x-half fp8 (scaled WS), h-half bf16 pre-scaled by AS*WS so
    # the shared PSUM group has a uniform 1/(AS*WS) descale
    wox8 = q8(_pack_w(f("Wo_w"), D)).reshape(JC, P, NQ1, 2, P)
    wou = np.ascontiguousarray(
        (_pack_w(f("Uo_w"), D) * (AS * WS)).astype(BF16))

    wr = np.stack([_pack_w(f(n + "_w"), D) for n in ("r1", "r2", "r3")]
                  ).astype(BF16)
    a2p = q8(np.ascontiguousarray(f("a2_w").reshape(KC1, P).T))  # [P, KC1]

    bias_vecs = []
    for g in ("Wi", "Wf", "Wo", "Wc", "Ws"):
        u = "U" + g[1]
        bias_vecs.append(f(g + "_b") + f(u + "_b"))
    bias_vecs += [f("a1_b") * RS, f("r1_b"), f("r2_b"), f("r3_b"),
                  np.full(D, f("a2_b")[0], np.float32)]
    # biasp[p, v*JC + j] = vec_v[j*128 + p]
    biasp = np.ascontiguousarray(
        np.stack(bias_vecs).reshape(10, JC, P).transpose(2, 0, 1).reshape(
            P, 10 * JC))

    x, h, c = f("x"), f("h_prev"), f("c_prev")
    shared = {"wg8": wg8, "wox8": wox8, "wou": wou, "wr": wr, "a2p": a2p,
              "biasp": biasp}
    in_maps = []
    for core in range(NCORES):
        sl = slice(core * BL, (core + 1) * BL)
        xhT = np.ascontiguousarray(
            np.concatenate([x[sl].T, h[sl].T], axis=0))  # [K2, BL]
        xh8 = np.ascontiguousarray(
            np.clip(xhT * AS, -240, 240).astype(F8NP).reshape(
                NQ2, 2, P, BL).transpose(0, 2, 1, 3))  # [NQ2, P, 2, BL]
        hTb = np.ascontiguousarray(h[sl].T.astype(BF16))
        cTc = np.ascontiguousarray(c[sl].T)
        in_maps.append({**shared, "xh8": xh8, "hTb": hTb, "cT": cTc})
    return in_maps


def _run(inputs, trace=False):
    from concourse.bass_utils import run_bass_kernel_spmd

    if "nc" not in _CACHE:
        _CACHE["nc"] = _build()
    nc = _CACHE["nc"]
    in_maps = _prepare(inputs)
    res = run_bass_kernel_spmd(nc, in_maps, core_ids=list(range(NCORES)),
                               trace=trace)
    h = np.empty((B, D), np.float32)
    c = np.empty((B, D), np.float32)
    for core in range(NCORES):
        o = res.results[core]["out"]  # [2, D, BL] bf16
        sl = slice(core * BL, (core + 1) * BL)
        h[sl] = o[0].T.astype(np.float32)
        c[sl] = o[1].T.astype(np.float32)
    return (h, c), res


def kernel(**inputs):
    (h, c), _ = _run(inputs, trace=False)
    return (h, c)
